# revision 6
# baseline (speedup 1.0000x reference)
"""Grouped MLP (MoE expert MLP) kernel for one TRN2 chip (8 NeuronCores).

Expert-parallel: expert e's tokens + weights go to core e (NE == n_cores == 8).
Per core computes out = gelu(x_e @ w1_e.T) @ w2_e with both matmuls on the
TensorEngine in bf16 (fp32 PSUM accumulation).

Layout: host pre-packs every tensor so that (a) the contraction dim lands on
SBUF partitions with zero device-side transposes and (b) every DMA moves
8KB+ contiguous per-partition lines (max HBM efficiency):
  matmul1: hT[f, t] = sum_h w1T[h, f] * xT[h, t]      (lhsT = w1T, rhs = xT)
  gelu    : on PSUM -> SBUF (ScalarE), output bf16
  matmul2: out[t, d] = sum_f hT[f, t] * w2[f, d]      (lhsT = hT, rhs = w2)

Perf structure (from NTFF trace analysis):
  - DMA issue order is consumption order: x chunk 0 first, then w1 in 512-col
    ffn groups, on the Sync HWDGE queue.  w2 goes on the Scalar HWDGE queue
    with triggers emitted mid-gelu-stream of chunk 0, so it never competes
    with the startup-critical x0/w1 transfers.
  - A burst of warm-up matmuls on scratch data keeps the PE busy (and its HAM
    clock gate at full 2.4 GHz) until the first real operands land (~13us).
  - Token chunks of 512 make every matmul N=512 (PSUM-bank-sized), which
    minimizes per-instruction NX overhead on the Tensor engine.
  - Output is written bf16 (host upcasts), halving output DMA traffic.
"""

import numpy as np
import ml_dtypes

NE = 8      # experts == cores
HID = 1024
FFN = 4096

_BF16 = ml_dtypes.bfloat16


def _install_axon_profile_hook():
    """Make run_bass_kernel_spmd(trace=True) usable in containers whose antenv
    package lacks axon_hooks. No-op if the real module is importable."""
    try:
        import antenv.axon_hooks  # noqa: F401
        return
    except ImportError:
        pass
    try:
        import sys
        import types

        import antenv  # noqa: F401

        mod = types.ModuleType("antenv.axon_hooks")
        mod._hook = None

        def set_axon_ntff_profile_hook(h):
            mod._hook = h

        def get_axon_ntff_profile_hook():
            return mod._hook

        mod.set_axon_ntff_profile_hook = set_axon_ntff_profile_hook
        mod.get_axon_ntff_profile_hook = get_axon_ntff_profile_hook
        sys.modules["antenv.axon_hooks"] = mod

        from trn_agent_boot.trn_boot import _ntff_profile_via_ctypes

        so_path = "/opt/axon/libaxon_pjrt.so"
        hook = _ntff_profile_via_ctypes(so_path)
        if hook is not None:
            mod._hook = hook
    except Exception:
        pass


def _build(T):
    """Build + compile the per-core Bass kernel for T tokens (multiple of 512)."""
    import concourse.mybir as mybir
    import concourse.tile as tile
    from concourse import bacc

    TC = 512            # token chunk (matmul1 moving free dim; one PSUM bank)
    HC = HID // 128     # 8 contraction chunks for matmul1
    FC = FFN // 128     # 32 f chunks
    NG = FFN // 512     # 8 ffn groups (w1 DMA granularity)
    NT = T // TC
    N_WARM = 40         # warm-up matmuls bridging until first operands land

    nc = bacc.Bacc("TRN2", target_bir_lowering=False, debug=False, num_devices=NE)
    # host-packed layouts: row blocks of 128 partitions, fully contiguous cols
    xt = nc.declare_dram_parameter(
        "xt", [NT * 128, HC * TC], mybir.dt.bfloat16, isOutput=False
    )
    w1t = nc.declare_dram_parameter(
        "w1t", [NG * 128, HC * 512], mybir.dt.bfloat16, isOutput=False
    )
    w2 = nc.declare_dram_parameter(
        "w2", [2 * 128, FC * 512], mybir.dt.bfloat16, isOutput=False
    )
    out = nc.declare_dram_parameter("out", [T, HID], mybir.dt.bfloat16, isOutput=True)

    with tile.TileContext(nc) as tc:
        with (
            tc.tile_pool(name="weights", bufs=1) as wpool,
            tc.tile_pool(name="xin", bufs=4) as xpool,
            tc.tile_pool(name="hmid", bufs=1) as hpool,
            tc.tile_pool(name="oout", bufs=4) as opool,
            tc.tile_pool(name="ph", bufs=3, space="PSUM") as ph_pool,
            tc.tile_pool(name="po", bufs=4, space="PSUM") as po_pool,
            tc.tile_pool(name="pwarm", bufs=1, space="PSUM") as pw_pool,
        ):
            # w1t_sb[p, g, c, fc]: lhsT for (c, fi=g*4+j) = [:, g, c, 128j:128j+128]
            w1t_sb = wpool.tile([128, NG, HC, 512], mybir.dt.bfloat16, tag="w1t")
            # w2_sb[p, d, c(=fi), dc]: rhs for (fi, d) = [:, d, fi, :]
            w2_sb = wpool.tile([128, 2, FC, 512], mybir.dt.bfloat16, tag="w2")
            scratch = wpool.tile([128, 640], mybir.dt.bfloat16, tag="scratch")

            # PE warm-up: one long accumulation on scratch data keeps the HAM
            # clock gate open while the first x / w1 transfers are in flight.
            nc.any.memset(scratch, 0)
            pw = pw_pool.tile([128, 512], mybir.dt.float32, tag="pw")
            for i in range(N_WARM):
                nc.tensor.matmul(
                    pw,
                    scratch[:, 0:128],
                    scratch[:, 128:640],
                    start=(i == 0),
                    stop=(i == N_WARM - 1),
                )

            x_sb = []
            for t in range(NT):
                x_sb.append(
                    xpool.tile(
                        [128, HC, TC], mybir.dt.bfloat16, tag="xt", name=f"xt{t}"
                    )
                )

            def dma_x(t):
                nc.sync.dma_start(out=x_sb[t], in_=xt[t * 128:(t + 1) * 128, :])

            def dma_w1(g):  # one 512-wide ffn group of w1t (1 MB, 8KB lines)
                nc.sync.dma_start(
                    out=w1t_sb[:, g], in_=w1t[g * 128:(g + 1) * 128, :]
                )

            def dma_w2(d):  # one 512-wide output-col half of w2 (4.2 MB)
                nc.scalar.dma_start(
                    out=w2_sb[:, d], in_=w2[d * 128:(d + 1) * 128, :]
                )

            # Sync-queue order == consumption order; w2 triggers are emitted
            # later (inside chunk 0's gelu stream) on the Scalar queue.
            dma_x(0)
            for g in range(NG):
                dma_w1(g)
            if NT > 1:
                dma_x(1)

            for t in range(NT):
                # prefetch x two chunks ahead; by issue time the ring buffer
                # it lands in has long been released, so the Sync queue never
                # blocks on it (which would stall output DMAs queued behind)
                if t + 2 < NT:
                    dma_x(t + 2)
                xt_sb = x_sb[t]
                h_sb = hpool.tile([128, FC, TC], mybir.dt.bfloat16, tag="h")
                for fi in range(FC):
                    g, j = divmod(fi, 4)
                    ph = ph_pool.tile([128, TC], mybir.dt.float32, tag="ph")
                    for c in range(HC):
                        nc.tensor.matmul(
                            ph,
                            w1t_sb[:, g, c, j * 128:(j + 1) * 128],
                            xt_sb[:, c, :],
                            start=(c == 0),
                            stop=(c == HC - 1),
                        )
                    nc.scalar.activation(
                        h_sb[:, fi, :], ph, mybir.ActivationFunctionType.Gelu
                    )
                    if t == 0 and fi == 4:
                        dma_w2(0)
                    if t == 0 and fi == 16:
                        dma_w2(1)
                for ti in range(TC // 128):
                    o_sb = opool.tile([128, HID], mybir.dt.bfloat16, tag="o")
                    for d in range(2):
                        po = po_pool.tile([128, 512], mybir.dt.float32, tag="po")
                        for fi in range(FC):
                            nc.tensor.matmul(
                                po,
                                h_sb[:, fi, ti * 128:(ti + 1) * 128],
                                w2_sb[:, d, fi, :],
                                start=(fi == 0),
                                stop=(fi == FC - 1),
                            )
                        nc.vector.tensor_copy(o_sb[:, d * 512:(d + 1) * 512], po)
                    row0 = t * TC + ti * 128
                    nc.sync.dma_start(out=out[row0:row0 + 128, :], in_=o_sb)

    nc.compile()
    return nc


_compiled = {}

LAST_RESULT = None


def kernel(x, tokens_per_expert, w1, w2):
    from concourse.bass_utils import run_bass_kernel_spmd

    _install_axon_profile_hook()

    x = np.asarray(x)
    w1 = np.asarray(w1)
    w2 = np.asarray(w2)
    tpe = np.asarray(tokens_per_expert).astype(np.int64)
    assert tpe.shape == (NE,)
    bounds = np.concatenate([[0], np.cumsum(tpe)])
    total = int(bounds[-1])
    maxt = max(int(tpe.max()), 1)
    T = ((maxt + 511) // 512) * 512
    NT = T // 512

    if T not in _compiled:
        _compiled[T] = _build(T)
    nc = _compiled[T]

    in_maps = []
    for e in range(NE):
        te = int(tpe[e])
        xe = np.zeros((T, HID), dtype=np.float32)
        xe[:te] = x[bounds[e]:bounds[e + 1]]
        # pack: row nt*128+p, col c*512+tt  <=  xT[c*128+p, nt*512+tt]
        xp = (
            xe.T.reshape(8, 128, NT, 512)
            .transpose(2, 1, 0, 3)
            .reshape(NT * 128, 8 * 512)
        )
        # pack: row g*128+p, col c*512+fc  <=  w1T[c*128+p, g*512+fc]
        w1p = (
            w1[e].T.reshape(8, 128, 8, 512)
            .transpose(2, 1, 0, 3)
            .reshape(8 * 128, 8 * 512)
        )
        # pack: row d*128+p, col c*512+dc  <=  w2[c*128+p, d*512+dc]
        w2p = (
            w2[e].reshape(32, 128, 2, 512)
            .transpose(2, 1, 0, 3)
            .reshape(2 * 128, 32 * 512)
        )
        in_maps.append(
            {
                "xt": np.ascontiguousarray(xp).astype(_BF16),
                "w1t": np.ascontiguousarray(w1p).astype(_BF16),
                "w2": np.ascontiguousarray(w2p).astype(_BF16),
            }
        )

    res = run_bass_kernel_spmd(nc, in_maps, core_ids=list(range(NE)))
    global LAST_RESULT
    LAST_RESULT = res

    out = np.zeros((x.shape[0], HID), dtype=np.float32)
    for e in range(NE):
        te = int(tpe[e])
        out[bounds[e]:bounds[e + 1]] = res.results[e]["out"][:te].astype(np.float32)
    assert total <= x.shape[0]
    return out


# revision 8
# speedup vs baseline: 1.0275x; 1.0275x over previous
"""Grouped MLP (MoE expert MLP) kernel for one TRN2 chip (8 NeuronCores).

Expert-parallel: expert e's tokens + weights go to core e (NE == n_cores == 8).
Per core computes out = gelu(x_e @ w1_e.T) @ w2_e with both matmuls on the
TensorEngine in bf16 (fp32 PSUM accumulation).

Layout: host pre-packs every tensor so that (a) the contraction dim lands on
SBUF partitions with zero device-side transposes and (b) every DMA moves
8KB+ contiguous per-partition lines (max HBM efficiency):
  matmul1: hT[f, t] = sum_h w1T[h, f] * xT[h, t]      (lhsT = w1T, rhs = xT)
  gelu    : on PSUM -> SBUF (ScalarE), output bf16
  matmul2: out[t, d] = sum_f hT[f, t] * w2[f, d]      (lhsT = hT, rhs = w2)

Perf structure (from NTFF trace analysis):
  - DMA issue order is consumption order: x chunk 0 first, then w1 in 512-col
    ffn groups, on the Sync HWDGE queue.  w2 goes on the Scalar HWDGE queue
    with triggers emitted mid-gelu-stream of chunk 0, so it never competes
    with the startup-critical x0/w1 transfers.
  - A burst of warm-up matmuls on scratch data keeps the PE busy (and its HAM
    clock gate at full 2.4 GHz) until the first real operands land (~13us).
  - Token chunks of 512 make every matmul N=512 (PSUM-bank-sized), which
    minimizes per-instruction NX overhead on the Tensor engine.
  - Output is written bf16 (host upcasts), halving output DMA traffic.
"""

import numpy as np
import ml_dtypes

NE = 8      # experts == cores
HID = 1024
FFN = 4096

_BF16 = ml_dtypes.bfloat16


def _install_axon_profile_hook():
    """Make run_bass_kernel_spmd(trace=True) usable in containers whose antenv
    package lacks axon_hooks. No-op if the real module is importable."""
    try:
        import antenv.axon_hooks  # noqa: F401
        return
    except ImportError:
        pass
    try:
        import sys
        import types

        import antenv  # noqa: F401

        mod = types.ModuleType("antenv.axon_hooks")
        mod._hook = None

        def set_axon_ntff_profile_hook(h):
            mod._hook = h

        def get_axon_ntff_profile_hook():
            return mod._hook

        mod.set_axon_ntff_profile_hook = set_axon_ntff_profile_hook
        mod.get_axon_ntff_profile_hook = get_axon_ntff_profile_hook
        sys.modules["antenv.axon_hooks"] = mod

        from trn_agent_boot.trn_boot import _ntff_profile_via_ctypes

        so_path = "/opt/axon/libaxon_pjrt.so"
        hook = _ntff_profile_via_ctypes(so_path)
        if hook is not None:
            mod._hook = hook
    except Exception:
        pass


def _build(T):
    """Build + compile the per-core Bass kernel for T tokens (multiple of 512)."""
    import concourse.mybir as mybir
    import concourse.tile as tile
    from concourse import bacc

    TC = 512            # token chunk (matmul1 moving free dim; one PSUM bank)
    HC = HID // 128     # 8 contraction chunks for matmul1
    FC = FFN // 128     # 32 f chunks
    NG = FFN // 512     # 8 ffn groups (w1 DMA granularity)
    NT = T // TC
    N_WARM = 40         # warm-up matmuls bridging until first operands land

    nc = bacc.Bacc("TRN2", target_bir_lowering=False, debug=False, num_devices=NE)
    # host-packed layouts: row blocks of 128 partitions, fully contiguous cols
    xt = nc.declare_dram_parameter(
        "xt", [NT * 128, HC * TC], mybir.dt.bfloat16, isOutput=False
    )
    w1t = nc.declare_dram_parameter(
        "w1t", [NG * 128, HC * 512], mybir.dt.bfloat16, isOutput=False
    )
    w2 = nc.declare_dram_parameter(
        "w2", [2 * 128, FC * 512], mybir.dt.bfloat16, isOutput=False
    )
    out = nc.declare_dram_parameter("out", [T, HID], mybir.dt.bfloat16, isOutput=True)

    with tile.TileContext(nc) as tc:
        with (
            tc.tile_pool(name="weights", bufs=1) as wpool,
            tc.tile_pool(name="xin", bufs=4) as xpool,
            tc.tile_pool(name="hmid", bufs=1) as hpool,
            tc.tile_pool(name="oout", bufs=4) as opool,
            tc.tile_pool(name="ph", bufs=3, space="PSUM") as ph_pool,
            tc.tile_pool(name="po", bufs=4, space="PSUM") as po_pool,
            tc.tile_pool(name="pwarm", bufs=1, space="PSUM") as pw_pool,
        ):
            # w1t_sb[p, g, c, fc]: lhsT for (c, fi=g*4+j) = [:, g, c, 128j:128j+128]
            w1t_sb = wpool.tile([128, NG, HC, 512], mybir.dt.bfloat16, tag="w1t")
            # w2_sb[p, d, c(=fi), dc]: rhs for (fi, d) = [:, d, fi, :]
            w2_sb = wpool.tile([128, 2, FC, 512], mybir.dt.bfloat16, tag="w2")
            scratch = wpool.tile([128, 640], mybir.dt.bfloat16, tag="scratch")

            # PE warm-up: one long accumulation on scratch data keeps the HAM
            # clock gate open while the first x / w1 transfers are in flight.
            nc.any.memset(scratch, 0)
            pw = pw_pool.tile([128, 512], mybir.dt.float32, tag="pw")
            for i in range(N_WARM):
                nc.tensor.matmul(
                    pw,
                    scratch[:, 0:128],
                    scratch[:, 128:640],
                    start=(i == 0),
                    stop=(i == N_WARM - 1),
                )

            x_sb = []
            for t in range(NT):
                x_sb.append(
                    xpool.tile(
                        [128, HC, TC], mybir.dt.bfloat16, tag="xt", name=f"xt{t}"
                    )
                )

            def dma_x(t):
                nc.sync.dma_start(out=x_sb[t], in_=xt[t * 128:(t + 1) * 128, :])

            def dma_w1(g):  # one 512-wide ffn group of w1t (1 MB, 8KB lines)
                nc.sync.dma_start(
                    out=w1t_sb[:, g], in_=w1t[g * 128:(g + 1) * 128, :]
                )

            def dma_w2(d):  # one 512-wide output-col half of w2 (4.2 MB)
                nc.sync.dma_start(
                    out=w2_sb[:, d], in_=w2[d * 128:(d + 1) * 128, :]
                )

            # Everything on the one Sync HWDGE queue, in consumption order:
            # the queue is FIFO, so x0 + the first w1 slices get 100% of the
            # (slowly ramping) HBM bandwidth and compute starts earliest.
            # Each later item still lands well before its consumer needs it.
            dma_x(0)
            for g in range(NG):
                dma_w1(g)
            dma_w2(0)
            if NT > 1:
                dma_x(1)
            dma_w2(1)
            for t in range(2, min(NT, 4)):
                dma_x(t)

            for t in range(NT):
                # for very long token counts, prefetch x two chunks ahead; the
                # ring buffer it lands in has long been released by then, so
                # the Sync queue never blocks on it (which would stall output
                # DMAs queued behind)
                if t + 2 >= 4 and t + 2 < NT:
                    dma_x(t + 2)
                xt_sb = x_sb[t]
                h_sb = hpool.tile([128, FC, TC], mybir.dt.bfloat16, tag="h")
                for fi in range(FC):
                    g, j = divmod(fi, 4)
                    ph = ph_pool.tile([128, TC], mybir.dt.float32, tag="ph")
                    for c in range(HC):
                        nc.tensor.matmul(
                            ph,
                            w1t_sb[:, g, c, j * 128:(j + 1) * 128],
                            xt_sb[:, c, :],
                            start=(c == 0),
                            stop=(c == HC - 1),
                        )
                    nc.scalar.activation(
                        h_sb[:, fi, :], ph, mybir.ActivationFunctionType.Gelu
                    )
                for ti in range(TC // 128):
                    o_sb = opool.tile([128, HID], mybir.dt.bfloat16, tag="o")
                    for d in range(2):
                        po = po_pool.tile([128, 512], mybir.dt.float32, tag="po")
                        for fi in range(FC):
                            nc.tensor.matmul(
                                po,
                                h_sb[:, fi, ti * 128:(ti + 1) * 128],
                                w2_sb[:, d, fi, :],
                                start=(fi == 0),
                                stop=(fi == FC - 1),
                            )
                        nc.vector.tensor_copy(o_sb[:, d * 512:(d + 1) * 512], po)
                    row0 = t * TC + ti * 128
                    nc.sync.dma_start(out=out[row0:row0 + 128, :], in_=o_sb)

    nc.compile()
    return nc


_compiled = {}

LAST_RESULT = None


def kernel(x, tokens_per_expert, w1, w2):
    from concourse.bass_utils import run_bass_kernel_spmd

    _install_axon_profile_hook()

    x = np.asarray(x)
    w1 = np.asarray(w1)
    w2 = np.asarray(w2)
    tpe = np.asarray(tokens_per_expert).astype(np.int64)
    assert tpe.shape == (NE,)
    bounds = np.concatenate([[0], np.cumsum(tpe)])
    total = int(bounds[-1])
    maxt = max(int(tpe.max()), 1)
    T = ((maxt + 511) // 512) * 512
    NT = T // 512

    if T not in _compiled:
        _compiled[T] = _build(T)
    nc = _compiled[T]

    in_maps = []
    for e in range(NE):
        te = int(tpe[e])
        xe = np.zeros((T, HID), dtype=np.float32)
        xe[:te] = x[bounds[e]:bounds[e + 1]]
        # pack: row nt*128+p, col c*512+tt  <=  xT[c*128+p, nt*512+tt]
        xp = (
            xe.T.reshape(8, 128, NT, 512)
            .transpose(2, 1, 0, 3)
            .reshape(NT * 128, 8 * 512)
        )
        # pack: row g*128+p, col c*512+fc  <=  w1T[c*128+p, g*512+fc]
        w1p = (
            w1[e].T.reshape(8, 128, 8, 512)
            .transpose(2, 1, 0, 3)
            .reshape(8 * 128, 8 * 512)
        )
        # pack: row d*128+p, col c*512+dc  <=  w2[c*128+p, d*512+dc]
        w2p = (
            w2[e].reshape(32, 128, 2, 512)
            .transpose(2, 1, 0, 3)
            .reshape(2 * 128, 32 * 512)
        )
        in_maps.append(
            {
                "xt": np.ascontiguousarray(xp).astype(_BF16),
                "w1t": np.ascontiguousarray(w1p).astype(_BF16),
                "w2": np.ascontiguousarray(w2p).astype(_BF16),
            }
        )

    res = run_bass_kernel_spmd(nc, in_maps, core_ids=list(range(NE)))
    global LAST_RESULT
    LAST_RESULT = res

    out = np.zeros((x.shape[0], HID), dtype=np.float32)
    for e in range(NE):
        te = int(tpe[e])
        out[bounds[e]:bounds[e + 1]] = res.results[e]["out"][:te].astype(np.float32)
    assert total <= x.shape[0]
    return out


# revision 9
# speedup vs baseline: 1.0400x; 1.0122x over previous
"""Grouped MLP (MoE expert MLP) kernel for one TRN2 chip (8 NeuronCores).

Expert-parallel: expert e's tokens + weights go to core e (NE == n_cores == 8).
Per core computes out = gelu(x_e @ w1_e.T) @ w2_e with both matmuls on the
TensorEngine in bf16 (fp32 PSUM accumulation).

Layout: host pre-packs every tensor so that (a) the contraction dim lands on
SBUF partitions with zero device-side transposes and (b) every DMA moves
multi-KB contiguous per-partition lines (max HBM efficiency):
  matmul1: hT[f, t] = sum_h w1T[h, f] * xT[h, t]      (lhsT = w1T, rhs = xT)
  gelu    : on PSUM -> SBUF (ScalarE), output bf16
  matmul2: out[t, d] = sum_f hT[f, t] * w2[f, d]      (lhsT = hT, rhs = w2)

Perf structure (from NTFF trace analysis):
  - One Sync-queue DMA stream in exact consumption order (x chunk 0, w1 in
    ffn groups sized 2,2,2,2,4x6 f-tiles, w2 halves, later x chunks), so the
    startup-critical transfers get 100% of the (slowly ramping) HBM bandwidth
    and the first real matmul starts ~13us in.
  - A short burst of warm-up matmuls on scratch data keeps the PE busy (and
    its HAM clock gate at full 2.4 GHz) until the first real operands land.
  - Token chunks of 512 make every matmul N=512 (PSUM-bank-sized), which
    minimizes per-instruction NX overhead on the Tensor engine.
  - Output is written bf16 (host upcasts) in 512-col halves, each CAST+DMA
    issued as soon as its PSUM accumulation stops (Vector handles d=0,
    Scalar handles d=1) to shorten the end-of-kernel serial chain.
"""

import numpy as np
import ml_dtypes

NE = 8      # experts == cores
HID = 1024
FFN = 4096

# w1 DMA group sizes in units of 128-wide f-tiles (sum must be FFN/128 == 32).
# Finer granularity up front lets matmul1 start as soon as ~1.5 MB has landed.
W1_GROUPS = [2, 2, 2, 2, 4, 4, 4, 4, 4, 4]

_BF16 = ml_dtypes.bfloat16


def _install_axon_profile_hook():
    """Make run_bass_kernel_spmd(trace=True) usable in containers whose antenv
    package lacks axon_hooks. No-op if the real module is importable."""
    try:
        import antenv.axon_hooks  # noqa: F401
        return
    except ImportError:
        pass
    try:
        import sys
        import types

        import antenv  # noqa: F401

        mod = types.ModuleType("antenv.axon_hooks")
        mod._hook = None

        def set_axon_ntff_profile_hook(h):
            mod._hook = h

        def get_axon_ntff_profile_hook():
            return mod._hook

        mod.set_axon_ntff_profile_hook = set_axon_ntff_profile_hook
        mod.get_axon_ntff_profile_hook = get_axon_ntff_profile_hook
        sys.modules["antenv.axon_hooks"] = mod

        from trn_agent_boot.trn_boot import _ntff_profile_via_ctypes

        so_path = "/opt/axon/libaxon_pjrt.so"
        hook = _ntff_profile_via_ctypes(so_path)
        if hook is not None:
            mod._hook = hook
    except Exception:
        pass


def _build(T):
    """Build + compile the per-core Bass kernel for T tokens (multiple of 512)."""
    import concourse.mybir as mybir
    import concourse.tile as tile
    from concourse import bacc

    TC = 512            # token chunk (matmul1 moving free dim; one PSUM bank)
    HC = HID // 128     # 8 contraction chunks for matmul1
    FC = FFN // 128     # 32 f chunks
    NT = T // TC
    N_WARM = 16         # warm-up matmuls bridging until first operands land

    assert sum(W1_GROUPS) == FC
    # flat per-partition column base of each w1 group; within a group the
    # layout is [c][j][fcol] with j the f-tile index local to the group
    gbase = np.concatenate([[0], np.cumsum([g * HC * 128 for g in W1_GROUPS])])
    # f-tile index fi -> (group, local j)
    fi2g = []
    for g, sz in enumerate(W1_GROUPS):
        for j in range(sz):
            fi2g.append((g, j))

    nc = bacc.Bacc("TRN2", target_bir_lowering=False, debug=False, num_devices=NE)
    # host-packed layouts: row blocks of 128 partitions, fully contiguous cols
    xt = nc.declare_dram_parameter(
        "xt", [NT * 128, HC * TC], mybir.dt.bfloat16, isOutput=False
    )
    w1t = nc.declare_dram_parameter(
        "w1t", [128, HC * FFN], mybir.dt.bfloat16, isOutput=False
    )
    w2 = nc.declare_dram_parameter(
        "w2", [2 * 128, FC * 512], mybir.dt.bfloat16, isOutput=False
    )
    out = nc.declare_dram_parameter("out", [T, HID], mybir.dt.bfloat16, isOutput=True)

    with tile.TileContext(nc) as tc:
        with (
            tc.tile_pool(name="weights", bufs=1) as wpool,
            tc.tile_pool(name="xin", bufs=4) as xpool,
            tc.tile_pool(name="hmid", bufs=1) as hpool,
            tc.tile_pool(name="oout", bufs=4) as opool,
            tc.tile_pool(name="ph", bufs=3, space="PSUM") as ph_pool,
            tc.tile_pool(name="po", bufs=4, space="PSUM") as po_pool,
            tc.tile_pool(name="pwarm", bufs=1, space="PSUM") as pw_pool,
        ):
            w1t_sb = wpool.tile([128, HC * FFN], mybir.dt.bfloat16, tag="w1t")
            # w2_sb[p, d, c(=fi), dc]: rhs for (fi, d) = [:, d, fi, :]
            w2_sb = wpool.tile([128, 2, FC, 512], mybir.dt.bfloat16, tag="w2")
            scratch = wpool.tile([128, 640], mybir.dt.bfloat16, tag="scratch")

            # PE warm-up: one long accumulation on scratch data keeps the HAM
            # clock gate open while the first x / w1 transfers are in flight.
            nc.any.memset(scratch, 0)
            pw = pw_pool.tile([128, 512], mybir.dt.float32, tag="pw")
            for i in range(N_WARM):
                nc.tensor.matmul(
                    pw,
                    scratch[:, 0:128],
                    scratch[:, 128:640],
                    start=(i == 0),
                    stop=(i == N_WARM - 1),
                )

            x_sb = []
            for t in range(NT):
                x_sb.append(
                    xpool.tile(
                        [128, HC, TC], mybir.dt.bfloat16, tag="xt", name=f"xt{t}"
                    )
                )

            def dma_x(t):
                nc.sync.dma_start(out=x_sb[t], in_=xt[t * 128:(t + 1) * 128, :])

            def dma_w1(g):  # one ffn group of w1t (contiguous cols both sides)
                c0, c1 = int(gbase[g]), int(gbase[g + 1])
                nc.sync.dma_start(out=w1t_sb[:, c0:c1], in_=w1t[:, c0:c1])

            def dma_w2(d):  # one 512-wide output-col half of w2 (4.2 MB)
                nc.sync.dma_start(
                    out=w2_sb[:, d], in_=w2[d * 128:(d + 1) * 128, :]
                )

            # Everything on the one Sync HWDGE queue, in consumption order:
            # the queue is FIFO, so x0 + the first w1 slices get 100% of the
            # (slowly ramping) HBM bandwidth and compute starts earliest.
            # Each later item still lands well before its consumer needs it.
            dma_x(0)
            for g in range(len(W1_GROUPS)):
                dma_w1(g)
            dma_w2(0)
            if NT > 1:
                dma_x(1)
            dma_w2(1)
            for t in range(2, min(NT, 4)):
                dma_x(t)

            for t in range(NT):
                # for very long token counts, prefetch x two chunks ahead; the
                # ring buffer it lands in has long been released by then, so
                # the Sync queue never blocks on it (which would stall output
                # DMAs queued behind)
                if t + 2 >= 4 and t + 2 < NT:
                    dma_x(t + 2)
                xt_sb = x_sb[t]
                h_sb = hpool.tile([128, FC, TC], mybir.dt.bfloat16, tag="h")
                for fi in range(FC):
                    g, j = fi2g[fi]
                    base = int(gbase[g])
                    ph = ph_pool.tile([128, TC], mybir.dt.float32, tag="ph")
                    for c in range(HC):
                        off = base + (c * W1_GROUPS[g] + j) * 128
                        nc.tensor.matmul(
                            ph,
                            w1t_sb[:, off:off + 128],
                            xt_sb[:, c, :],
                            start=(c == 0),
                            stop=(c == HC - 1),
                        )
                    nc.scalar.activation(
                        h_sb[:, fi, :], ph, mybir.ActivationFunctionType.Gelu
                    )
                for ti in range(TC // 128):
                    row0 = t * TC + ti * 128
                    for d in range(2):
                        po = po_pool.tile([128, 512], mybir.dt.float32, tag="po")
                        for fi in range(FC):
                            nc.tensor.matmul(
                                po,
                                h_sb[:, fi, ti * 128:(ti + 1) * 128],
                                w2_sb[:, d, fi, :],
                                start=(fi == 0),
                                stop=(fi == FC - 1),
                            )
                        o_sb = opool.tile(
                            [128, 512], mybir.dt.bfloat16, tag="o", name=f"o{d}"
                        )
                        # d=0 converts on Vector, d=1 on Scalar: the two halves
                        # flush in parallel and the final serial chain after
                        # the last matmul is one 512-col CAST + one small DMA
                        if d == 0:
                            nc.vector.tensor_copy(o_sb, po)
                        else:
                            nc.scalar.activation(
                                o_sb, po, mybir.ActivationFunctionType.Copy
                            )
                        nc.sync.dma_start(
                            out=out[row0:row0 + 128, d * 512:(d + 1) * 512],
                            in_=o_sb,
                        )

    nc.compile()
    return nc


_compiled = {}

LAST_RESULT = None


def _pack_w1(w1e):
    """row p, cols: concat over groups g of [c][j][fcol] <= w1T[c*128+p, fi*128+fc]."""
    w1T = w1e.T  # [HID, FFN]
    blocks = []
    a = 0
    for sz in W1_GROUPS:
        blk = w1T[:, a * 128:(a + sz) * 128]        # [1024, sz*128]
        blk = blk.reshape(8, 128, sz * 128).transpose(1, 0, 2).reshape(128, -1)
        blocks.append(blk)
        a += sz
    return np.concatenate(blocks, axis=1)


def kernel(x, tokens_per_expert, w1, w2):
    from concourse.bass_utils import run_bass_kernel_spmd

    _install_axon_profile_hook()

    x = np.asarray(x)
    w1 = np.asarray(w1)
    w2 = np.asarray(w2)
    tpe = np.asarray(tokens_per_expert).astype(np.int64)
    assert tpe.shape == (NE,)
    bounds = np.concatenate([[0], np.cumsum(tpe)])
    total = int(bounds[-1])
    maxt = max(int(tpe.max()), 1)
    T = ((maxt + 511) // 512) * 512
    NT = T // 512

    if T not in _compiled:
        _compiled[T] = _build(T)
    nc = _compiled[T]

    in_maps = []
    for e in range(NE):
        te = int(tpe[e])
        xe = np.zeros((T, HID), dtype=np.float32)
        xe[:te] = x[bounds[e]:bounds[e + 1]]
        # pack: row nt*128+p, col c*512+tt  <=  xT[c*128+p, nt*512+tt]
        xp = (
            xe.T.reshape(8, 128, NT, 512)
            .transpose(2, 1, 0, 3)
            .reshape(NT * 128, 8 * 512)
        )
        # pack: row d*128+p, col c*512+dc  <=  w2[c*128+p, d*512+dc]
        w2p = (
            w2[e].reshape(32, 128, 2, 512)
            .transpose(2, 1, 0, 3)
            .reshape(2 * 128, 32 * 512)
        )
        in_maps.append(
            {
                "xt": np.ascontiguousarray(xp).astype(_BF16),
                "w1t": np.ascontiguousarray(_pack_w1(w1[e])).astype(_BF16),
                "w2": np.ascontiguousarray(w2p).astype(_BF16),
            }
        )

    res = run_bass_kernel_spmd(nc, in_maps, core_ids=list(range(NE)))
    global LAST_RESULT
    LAST_RESULT = res

    out = np.zeros((x.shape[0], HID), dtype=np.float32)
    for e in range(NE):
        te = int(tpe[e])
        out[bounds[e]:bounds[e + 1]] = res.results[e]["out"][:te].astype(np.float32)
    assert total <= x.shape[0]
    return out
